# revision 42
# baseline (speedup 1.0000x reference)
"""ErnieLayout self-attention on 8 Trainium2 NeuronCores (Bass/Tile).

Problem shapes (hardcoded): B=4, S=1024, H=768, NH=12, HD=64.
Sharding: core c -> (batch b = c//2, head-half hh = c%2, i.e. 6 heads).
Each core computes attention for its 6 heads of one batch element and
writes the [S, 384] column slice of that batch's output.

Per-core algorithm (mixed precision, scores kept transposed):
  setup:  X and W cast to fp16 (DVE), transposed on the PE (fp16 path);
          Q^T = (Wq_s @ X^T + bq)/8, K^T = Wk_s @ X^T + bk   (fp16 matmuls,
          fp32 PSUM accumulate, fp16 output tiles)
          V = X @ Wv_s^T (+ bv via DVE broadcast add), stored fp16 with a
          ones column appended (col 64 -> softmax denominator for free)
  per (head, ktile, q-chunk):
          psum[k=128, q=512] = K^T.T @ Q^T               (fp16, 1 cyc/row)
          psum += rel12[q,ktile]^T via matmul(lhsT=rel12_f16, rhs=I_f16)
          pT = exp(psum + maskbias[k]) -> fp16   (ACT per-partition bias;
          masked keys get FLT_MIN so exp underflows to exactly 0, matching
          the reference's FLT_MIN replacement; no row-max needed, scores
          are O(10))
  per head (PV flipped so V is the stationary operand and the fp16 pT
  strips stream at N=512):
          ctx^T[d|1, q-chunk] += V_aug[kt].T @ pT[kt]  over kt
          ctx^T -> SBUF (ACT), back-transposed on the PE (fp32, exact),
          out[q, h*64+d] = ctx[q, d] * (1 / ctx[q, 64])  (DVE recip + ACT
          scale; the finalize of head h is emitted inside head h+1's loop
          so the in-order PE stream never stalls on it)

rel12 = rel_pos + rel_2d_pos is one DVE pass (fp32 in, fp16 out) over
[128, 1024] strips streamed continuously from t=0 (the rel pools are
allocated before the phase-1 pools so their SBUF is disjoint).
Precision: fp16 carries 10 mantissa bits -> final rel err ~1e-3.
"""

import os
import sys

import numpy as np

for _p in ("/opt/trn_rl_repo",):
    if _p not in sys.path and os.path.isdir(_p):
        sys.path.append(_p)

import concourse.bass as bass
import concourse.mybir as mybir
import concourse.tile as tile
from concourse import bacc
from concourse.bass_utils import run_bass_kernel_spmd
from concourse.masks import make_identity

F32 = mybir.dt.float32
F32R = mybir.dt.float32r
F16 = mybir.dt.float16
I32 = mybir.dt.int32
AF = mybir.ActivationFunctionType
NEG = float(np.finfo(np.float32).min)

P = 128
S = 1024
NH = 6        # heads per core
HD = 64
HIN = 768     # model dim (contraction for projections)
HOUT = NH * HD  # 384, per-core projection width
KT = S // P   # 8 key tiles
QT = S // P   # 8 query tiles
VW = HD + 1   # 65: V columns + ones column


def _build_kernel_body(tc, aps):
    import contextlib

    nc = tc.nc
    x_ap = aps["x"]
    mask_ap = aps["mask"]
    rel1_ap = aps["rel1"]
    rel2_ap = aps["rel2"]
    out_ap = aps["out"]

    with contextlib.ExitStack() as ctx:
        const = ctx.enter_context(tc.tile_pool(name="const", bufs=1))

        ident = const.tile([P, P], F16)
        make_identity(nc, ident)
        ident32 = const.tile([P, P], F32)
        nc.vector.tensor_copy(ident32[:], ident[:])


        # long-lived tensors
        qt_pool = ctx.enter_context(tc.tile_pool(name="qT", bufs=3))
        kt_pool = ctx.enter_context(tc.tile_pool(name="kT", bufs=3))
        v_pool = ctx.enter_context(tc.tile_pool(name="v", bufs=8))

        qT = [qt_pool.tile([P, S], F16, tag="qT", name=f"qT{i}") for i in range(3)]
        kT = [kt_pool.tile([P, S], F16, tag="kT", name=f"kT{i}") for i in range(3)]
        v_tiles = [
            v_pool.tile([P, NH, VW], F16, tag="v", name=f"v{i}") for i in range(8)
        ]

        # rel stream pools first: their SBUF is disjoint from phase-1 pools,
        # so rel DMA + DVE adds run from t=0 and deep fp16 buffering keeps
        # the DMA queues fed across head boundaries.
        r1_pool = ctx.enter_context(tc.tile_pool(name="r1", bufs=5))
        r2_pool = ctx.enter_context(tc.tile_pool(name="r2", bufs=5))
        rbf_pool = ctx.enter_context(tc.tile_pool(name="rbf", bufs=32))

        # ---------------- phase 1: load, cast, transpose, project ----------
        with contextlib.ExitStack() as ph1:
            xload = ph1.enter_context(tc.tile_pool(name="xload", bufs=2))
            wload = ph1.enter_context(tc.tile_pool(name="wload", bufs=2))
            x16_pool = ph1.enter_context(tc.tile_pool(name="x16", bufs=8))
            w16_pool = ph1.enter_context(tc.tile_pool(name="w16", bufs=4))
            xt_pool = ph1.enter_context(tc.tile_pool(name="xT", bufs=6))
            wt_pool = ph1.enter_context(tc.tile_pool(name="wT", bufs=18))
            psum1 = ph1.enter_context(tc.tile_pool(name="psum1", bufs=3, space="PSUM"))
            psum1b = ph1.enter_context(
                tc.tile_pool(name="psum1b", bufs=2, space="PSUM")
            )

            # X tiles [128, 768] -> fp16
            x16 = []
            for t in range(8):
                xt_ = xload.tile([P, HIN], F32, tag="x")
                nc.sync.dma_start(xt_[:], x_ap[t * P:(t + 1) * P, :])
                x16_t = x16_pool.tile([P, HIN], F16, tag="x16", name=f"x16_{t}")
                nc.vector.tensor_copy(x16_t[:], xt_[:])
                x16.append(x16_t)

            # mask bias and projection biases: emitted after the X loads so
            # their many-descriptor gather DMAs and DVE ops stay off the
            # startup critical path (only needed from the first exp / proj)
            mask_i = const.tile([P, KT], I32)
            nc.sync.dma_start(mask_i[:], mask_ap.rearrange("(a p) -> p a", p=P))
            maskb = const.tile([P, KT], F32)
            nc.vector.tensor_copy(maskb[:], mask_i[:])
            nc.vector.tensor_scalar_mul(maskb[:], maskb[:], NEG)
            bias_sb = {}
            for wname in ("q", "k"):
                bt = const.tile([P, 3], F32, tag=f"b{wname}")
                nc.sync.dma_start(
                    bt[:], aps[f"b{wname}"].rearrange("(a p) -> p a", p=P)
                )
                if wname == "q":
                    nc.vector.tensor_scalar_mul(bt[:], bt[:], 0.125)
                bias_sb[wname] = bt
            bv_bc = const.tile([P, NH, HD], F32)
            nc.sync.dma_start(
                bv_bc[:],
                aps["bv"].rearrange("(h d) -> h d", d=HD)[None].to_broadcast(
                    (P, NH, HD)
                ),
            )

            # X^T: 6 fp16 tiles [128, 1024] (h-chunk on partitions)
            xT = []
            for hc in range(6):
                pt = psum1.tile([P, S], F16, tag="xtp")  # 1 bank (fp16)
                for t in range(8):
                    nc.tensor.transpose(
                        pt[:, t * P:(t + 1) * P],
                        x16[t][:, hc * P:(hc + 1) * P],
                        ident[:],
                    )
                xt_t = xt_pool.tile([P, S], F16, tag="xT")
                nc.scalar.copy(xt_t[:], pt[:])
                xT.append(xt_t)

            # W^T slices (fp16): wT[(w, hc)] = [128, 384]
            wT = {}
            for wname in ("q", "k", "v"):
                w_ap = aps[f"w{wname}"]
                w16s = []
                for d in range(3):
                    wt_ = wload.tile([P, HIN], F32, tag="wload")
                    nc.sync.dma_start(wt_[:], w_ap[d * P:(d + 1) * P, :])
                    w16_t = w16_pool.tile(
                        [P, HIN], F16, tag="w16", name=f"w16{wname}_{d}"
                    )
                    nc.vector.tensor_copy(w16_t[:], wt_[:])
                    w16s.append(w16_t)
                for hc in range(6):
                    pw = psum1b.tile([P, 512], F16, tag="ps1b", name="pw")[:, :HOUT]
                    for d in range(3):
                        nc.tensor.transpose(
                            pw[:, d * P:(d + 1) * P],
                            w16s[d][:, hc * P:(hc + 1) * P],
                            ident[:],
                        )
                    wt_t = wt_pool.tile([P, HOUT], F16, tag="wT")
                    nc.scalar.copy(wt_t[:], pw[:])
                    wT[(wname, hc)] = wt_t

            # Q^T, K^T projections: fp16 matmuls, fp32 PSUM, fp32r output
            for wname, dest, scale in (("q", qT, 0.125), ("k", kT, 1.0)):
                for d in range(3):
                    for tch in range(2):
                        pp = psum1b.tile([P, 512], F32, tag="projp")
                        for hc in range(6):
                            nc.tensor.matmul(
                                pp[:],
                                wT[(wname, hc)][:, d * P:(d + 1) * P],
                                xT[hc][:, tch * 512:(tch + 1) * 512],
                                start=(hc == 0),
                                stop=(hc == 5),
                            )
                        nc.scalar.activation(
                            dest[d][:, tch * 512:(tch + 1) * 512],
                            pp[:],
                            AF.Identity,
                            bias=bias_sb[wname][:, d:d + 1],
                            scale=scale,
                        )

            # V projection: out [t-tile 128, 384] fp16 + ones column
            for t in range(8):
                pv = psum1b.tile([P, 512], F32, tag="projp", name="pv")[:, :HOUT]
                for hc in range(6):
                    nc.tensor.matmul(
                        pv[:],
                        xT[hc][:, t * P:(t + 1) * P],
                        wT[("v", hc)][:],
                        start=(hc == 0),
                        stop=(hc == 5),
                    )
                nc.vector.memset(v_tiles[t][:], 1.0)
                # copy + bias add (bv broadcast along partitions)
                nc.vector.tensor_add(
                    v_tiles[t][:, :, 0:HD],
                    pv[:].rearrange("p (h d) -> p h d", d=HD),
                    bv_bc[:],
                )

        # ---------------- phase 2: attention per head ----------------
        out_pool = ctx.enter_context(tc.tile_pool(name="outst", bufs=8))
        out_stage = [
            out_pool.tile([P, HOUT], F32, tag="outst", name=f"outst{i}")
            for i in range(8)
        ]
        pt_pool = ctx.enter_context(tc.tile_pool(name="pT", bufs=18))
        fin_pool = ctx.enter_context(tc.tile_pool(name="fin", bufs=4))
        spsum = ctx.enter_context(tc.tile_pool(name="spsum", bufs=4, space="PSUM"))
        vpsum = ctx.enter_context(tc.tile_pool(name="vpsum", bufs=4, space="PSUM"))
        ctt_pool = ctx.enter_context(tc.tile_pool(name="ctt", bufs=2))

        def emit_finalize(h, ctxT_ps):
            """Epilogue for head h: copy ctx^T out of PSUM, back-transpose to
            [q, 65], divide by the denominator. Deferred one head so the
            in-order PE stream never stalls waiting on the ACT copies."""
            ctxT_sb = [None, None]
            for qch in range(2):
                t_ = ctt_pool.tile([VW, 512], F32, tag="ctxT_sb",
                                   name=f"ctxTs{h}_{qch}")
                nc.scalar.copy(t_[:], ctxT_ps[qch][:])
                ctxT_sb[qch] = t_
            ctx_ps = [
                vpsum.tile([P, 512], F32, tag="ctxT", name=f"ctx{h}_{i}")
                for i in range(2)
            ]
            # all PE transposes first, then all DVE reads: avoids the
            # per-slot PE-write/DVE-read same-bank ping-pong serialization
            for qt in range(QT):
                cp = ctx_ps[qt // 4]
                sl = (qt % 4) * VW
                nc.tensor.transpose(
                    cp[:, sl:sl + VW],
                    ctxT_sb[qt // 4][:, (qt % 4) * P:(qt % 4 + 1) * P],
                    ident32[:VW, :VW],
                )
            for qt in range(QT):
                cp = ctx_ps[qt // 4]
                sl = (qt % 4) * VW
                rc = fin_pool.tile([P, 1], F32, tag="recip")
                nc.vector.reciprocal(rc[:], cp[:, sl + HD:sl + HD + 1])
                nc.scalar.activation(
                    out_stage[qt][:, h * HD:(h + 1) * HD],
                    cp[:, sl:sl + HD],
                    AF.Identity,
                    scale=rc[:],
                )

        pending_fin = None
        for h in range(NH):
            # rel12 = rel1 + rel2 -> fp16, eight strip tiles [128, 1024]
            strips = []
            for qq in range(8):
                r1 = r1_pool.tile([P, S], F32, tag="r1")
                nc.sync.dma_start(
                    r1[:],
                    rel1_ap[h].rearrange("(qt p) k -> p qt k", p=P)[:, qq, :],
                )
                r2 = r2_pool.tile([P, S], F32, tag="r2")
                nc.sync.dma_start(
                    r2[:],
                    rel2_ap[h].rearrange("(qt p) k -> p qt k", p=P)[:, qq, :],
                )
                rb = rbf_pool.tile([P, S], F16, tag="rbf", name=f"rbf{h}_{qq}")
                nc.vector.tensor_add(rb[:], r1[:], r2[:])
                strips.append(rb)

            dt, rem = divmod(h, 2)
            d0 = rem * HD
            qTh = qT[dt][d0:d0 + HD, :]
            kTh = kT[dt][d0:d0 + HD, :]

            pT_strips = []
            for kt in range(KT):
                pT_strip = pt_pool.tile([P, S], F16, tag="pT", name=f"pT{h}_{kt}")
                pT_strips.append(pT_strip)
                for qch in range(2):
                    ps = spsum.tile([P, 512], F32, tag="sT")
                    # qk^T (fp32r: full-rate single-pass matmul)
                    nc.tensor.matmul(
                        ps[:],
                        kTh[:, kt * P:(kt + 1) * P],
                        qTh[:, qch * 512:(qch + 1) * 512],
                        start=True,
                        stop=False,
                    )
                    # += rel12^T (transposing adds via fp16 identity rhs)
                    for j in range(4):
                        qt = qch * 4 + j
                        nc.tensor.matmul(
                            ps[:, j * P:(j + 1) * P],
                            strips[qt][:, kt * P:(kt + 1) * P],
                            ident[:],
                            start=False,
                            stop=(j == 3),
                        )
                    # exp(scores + mask bias) -> fp16 probs
                    nc.scalar.activation(
                        pT_strip[:, qch * 512:(qch + 1) * 512],
                        ps[:],
                        AF.Exp,
                        bias=maskb[:, kt:kt + 1],
                        scale=1.0,
                    )
                if kt == 0 and pending_fin is not None:
                    emit_finalize(*pending_fin)
                    pending_fin = None

            # PV flipped: ctx^T[d|1, q] = V_aug.T @ P^T, accumulated over kt.
            # lhsT = V_aug tile (65 cols), rhs = pT strip (N=512 fp16) --
            # 16 big matmuls per head instead of 64 small ones. Row 64 of
            # ctx^T is the softmax denominator (ones column of V_aug).
            ctxT_ps = [
                vpsum.tile([VW, 512], F32, tag="ctxT", name=f"ctxT{h}_{i}")
                for i in range(2)
            ]
            for qch in range(2):
                for kt in range(KT):
                    nc.tensor.matmul(
                        ctxT_ps[qch][:],
                        v_tiles[kt][:, h, :],
                        pT_strips[kt][:, qch * 512:(qch + 1) * 512],
                        start=(kt == 0),
                        stop=(kt == KT - 1),
                    )
            pending_fin = (h, ctxT_ps)

        emit_finalize(*pending_fin)

        for qt in range(QT):
            nc.sync.dma_start(out_ap[qt * P:(qt + 1) * P, :], out_stage[qt][:])



def build_program():
    """Build and compile the per-core Bass program. Returns nc."""
    nc = bacc.Bacc(
        "TRN2",
        target_bir_lowering=False,
        debug=False,
        num_devices=8,
    )
    aps = {
        "x": nc.dram_tensor("x", [S, HIN], F32, kind="ExternalInput").ap(),
        "mask": nc.dram_tensor("mask", [S], I32, kind="ExternalInput").ap(),
        "rel1": nc.dram_tensor("rel1", [NH, S, S], F32, kind="ExternalInput").ap(),
        "rel2": nc.dram_tensor("rel2", [NH, S, S], F32, kind="ExternalInput").ap(),
        "wq": nc.dram_tensor("wq", [HOUT, HIN], F32, kind="ExternalInput").ap(),
        "wk": nc.dram_tensor("wk", [HOUT, HIN], F32, kind="ExternalInput").ap(),
        "wv": nc.dram_tensor("wv", [HOUT, HIN], F32, kind="ExternalInput").ap(),
        "bq": nc.dram_tensor("bq", [HOUT], F32, kind="ExternalInput").ap(),
        "bk": nc.dram_tensor("bk", [HOUT], F32, kind="ExternalInput").ap(),
        "bv": nc.dram_tensor("bv", [HOUT], F32, kind="ExternalInput").ap(),
        "out": nc.dram_tensor("out", [S, HOUT], F32, kind="ExternalOutput").ap(),
    }
    with tile.TileContext(nc) as tc:
        _build_kernel_body(tc, aps)
    nc.compile()
    return nc


def make_in_maps(inputs):
    """Slice full inputs into the 8 per-core input maps."""
    hs = np.ascontiguousarray(np.asarray(inputs["hidden_states"], np.float32))
    am = np.asarray(inputs["attention_mask"]).astype(np.int32)
    rel1 = np.asarray(inputs["rel_pos"], np.float32)
    rel2 = np.asarray(inputs["rel_2d_pos"], np.float32)
    ws = {k: np.asarray(inputs["W" + k[-1]], np.float32) for k in ("wq", "wk", "wv")}
    bs = {k: np.asarray(inputs["b" + k[-1]], np.float32) for k in ("bq", "bk", "bv")}

    in_maps = []
    for c in range(8):
        b, hh = divmod(c, 2)
        hsl = slice(hh * NH, (hh + 1) * NH)
        csl = slice(hh * HOUT, (hh + 1) * HOUT)
        m = {
            "x": np.ascontiguousarray(hs[b]),
            "mask": np.ascontiguousarray(am[b, 0, 0]),
            "rel1": np.ascontiguousarray(rel1[b, hsl]),
            "rel2": np.ascontiguousarray(rel2[b, hsl]),
        }
        for k in ("wq", "wk", "wv"):
            m[k] = np.ascontiguousarray(ws[k][csl])
        for k in ("bq", "bk", "bv"):
            m[k] = np.ascontiguousarray(bs[k][csl])
        in_maps.append(m)
    return in_maps


def gather_output(results):
    out = np.empty((4, S, HIN), np.float32)
    for c in range(8):
        b, hh = divmod(c, 2)
        out[b, :, hh * HOUT:(hh + 1) * HOUT] = results[c]["out"]
    return out


_NC_CACHE = []


def kernel(**inputs):
    if not _NC_CACHE:
        _NC_CACHE.append(build_program())
    nc = _NC_CACHE[0]
    in_maps = make_in_maps(inputs)
    res = run_bass_kernel_spmd(nc, in_maps, list(range(8)))
    return gather_output(res.results)


# revision 43
# speedup vs baseline: 1.0540x; 1.0540x over previous
"""ErnieLayout self-attention on 8 Trainium2 NeuronCores (Bass/Tile).

Problem shapes (hardcoded): B=4, S=1024, H=768, NH=12, HD=64.
Sharding: core c -> (batch b = c//2, head-half hh = c%2, i.e. 6 heads).
Each core computes attention for its 6 heads of one batch element and
writes the [S, 384] column slice of that batch's output.

Per-core algorithm (mixed precision, scores kept transposed):
  setup:  X and W cast to fp16 (DVE), transposed on the PE (fp16 path);
          Q^T = (Wq_s @ X^T + bq)/8, K^T = Wk_s @ X^T + bk   (fp16 matmuls,
          fp32 PSUM accumulate, fp16 output tiles)
          V = X @ Wv_s^T (+ bv via DVE broadcast add), stored fp16 with a
          ones column appended (col 64 -> softmax denominator for free)
  per (head, ktile, q-chunk):
          psum[k=128, q=512] = K^T.T @ Q^T               (fp16, 1 cyc/row)
          psum += rel12[q,ktile]^T via matmul(lhsT=rel12_f16, rhs=I_f16)
          pT = exp(psum + maskbias[k]) -> fp16   (ACT per-partition bias;
          masked keys get FLT_MIN so exp underflows to exactly 0, matching
          the reference's FLT_MIN replacement; no row-max needed, scores
          are O(10))
  per head (PV flipped so V is the stationary operand and the fp16 pT
  strips stream at N=512):
          ctx^T[d|1, q-chunk] += V_aug[kt].T @ pT[kt]  over kt
          ctx^T -> SBUF (ACT), back-transposed on the PE (fp32, exact),
          out[q, h*64+d] = ctx[q, d] * (1 / ctx[q, 64])  (DVE recip + ACT
          scale; the finalize of head h is emitted inside head h+1's loop
          so the in-order PE stream never stalls on it)

rel12 = rel_pos + rel_2d_pos is one DVE pass (fp32 in, fp16 out) over
[128, 1024] strips streamed continuously from t=0 (the rel pools are
allocated before the phase-1 pools so their SBUF is disjoint).
Precision: fp16 carries 10 mantissa bits -> final rel err ~1e-3.
"""

import os
import sys

import numpy as np

for _p in ("/opt/trn_rl_repo",):
    if _p not in sys.path and os.path.isdir(_p):
        sys.path.append(_p)

import concourse.bass as bass
import concourse.mybir as mybir
import concourse.tile as tile
from concourse import bacc
from concourse.bass_utils import run_bass_kernel_spmd
from concourse.masks import make_identity

F32 = mybir.dt.float32
F32R = mybir.dt.float32r
F16 = mybir.dt.float16
I32 = mybir.dt.int32
AF = mybir.ActivationFunctionType
NEG = float(np.finfo(np.float32).min)

P = 128
S = 1024
NH = 6        # heads per core
HD = 64
HIN = 768     # model dim (contraction for projections)
HOUT = NH * HD  # 384, per-core projection width
KT = S // P   # 8 key tiles
QT = S // P   # 8 query tiles
VW = HD + 1   # 65: V columns + ones column


def _build_kernel_body(tc, aps):
    import contextlib

    nc = tc.nc
    x_ap = aps["x"]
    mask_ap = aps["mask"]
    rel1_ap = aps["rel1"]
    rel2_ap = aps["rel2"]
    out_ap = aps["out"]

    with contextlib.ExitStack() as ctx:
        const = ctx.enter_context(tc.tile_pool(name="const", bufs=1))

        ident = const.tile([P, P], F16)
        make_identity(nc, ident)
        ident32 = const.tile([P, P], F32)
        nc.vector.tensor_copy(ident32[:], ident[:])


        # long-lived tensors
        qt_pool = ctx.enter_context(tc.tile_pool(name="qT", bufs=3))
        kt_pool = ctx.enter_context(tc.tile_pool(name="kT", bufs=3))
        v_pool = ctx.enter_context(tc.tile_pool(name="v", bufs=8))

        qT = [qt_pool.tile([P, S], F16, tag="qT", name=f"qT{i}") for i in range(3)]
        kT = [kt_pool.tile([P, S], F16, tag="kT", name=f"kT{i}") for i in range(3)]
        v_tiles = [
            v_pool.tile([P, NH, VW], F16, tag="v", name=f"v{i}") for i in range(8)
        ]

        # rel stream pools first: their SBUF is disjoint from phase-1 pools,
        # so rel DMA + DVE adds run from t=0 and deep fp16 buffering keeps
        # the DMA queues fed across head boundaries.
        r1_pool = ctx.enter_context(tc.tile_pool(name="r1", bufs=5))
        r2_pool = ctx.enter_context(tc.tile_pool(name="r2", bufs=5))
        rbf_pool = ctx.enter_context(tc.tile_pool(name="rbf", bufs=32))

        # ---------------- phase 1: load, cast, transpose, project ----------
        with contextlib.ExitStack() as ph1:
            xload = ph1.enter_context(tc.tile_pool(name="xload", bufs=2))
            wload = ph1.enter_context(tc.tile_pool(name="wload", bufs=2))
            x16_pool = ph1.enter_context(tc.tile_pool(name="x16", bufs=8))
            w16_pool = ph1.enter_context(tc.tile_pool(name="w16", bufs=4))
            xt_pool = ph1.enter_context(tc.tile_pool(name="xT", bufs=6))
            wt_pool = ph1.enter_context(tc.tile_pool(name="wT", bufs=18))
            psum1 = ph1.enter_context(tc.tile_pool(name="psum1", bufs=3, space="PSUM"))
            psum1b = ph1.enter_context(
                tc.tile_pool(name="psum1b", bufs=2, space="PSUM")
            )

            # X tiles [128, 768] -> fp16
            x16 = []
            for t in range(8):
                xt_ = xload.tile([P, HIN], F32, tag="x")
                nc.sync.dma_start(xt_[:], x_ap[t * P:(t + 1) * P, :])
                x16_t = x16_pool.tile([P, HIN], F16, tag="x16", name=f"x16_{t}")
                nc.vector.tensor_copy(x16_t[:], xt_[:])
                x16.append(x16_t)

            # mask bias and projection biases: emitted after the X loads so
            # their many-descriptor gather DMAs and DVE ops stay off the
            # startup critical path (only needed from the first exp / proj)
            mask_i = const.tile([P, KT], I32)
            nc.sync.dma_start(mask_i[:], mask_ap.rearrange("(a p) -> p a", p=P))
            maskb = const.tile([P, KT], F32)
            nc.vector.tensor_copy(maskb[:], mask_i[:])
            nc.vector.tensor_scalar_mul(maskb[:], maskb[:], NEG)
            bias_sb = {}
            for wname in ("q", "k"):
                bt = const.tile([P, 3], F32, tag=f"b{wname}")
                nc.sync.dma_start(
                    bt[:], aps[f"b{wname}"].rearrange("(a p) -> p a", p=P)
                )
                if wname == "q":
                    nc.vector.tensor_scalar_mul(bt[:], bt[:], 0.125)
                bias_sb[wname] = bt
            bv_bc = const.tile([P, NH, HD], F32)
            nc.sync.dma_start(
                bv_bc[:],
                aps["bv"].rearrange("(h d) -> h d", d=HD)[None].to_broadcast(
                    (P, NH, HD)
                ),
            )

            # X^T: 6 fp16 tiles [128, 1024] (h-chunk on partitions)
            xT = []
            for hc in range(6):
                pt = psum1.tile([P, S], F16, tag="xtp")  # 1 bank (fp16)
                for t in range(8):
                    nc.tensor.transpose(
                        pt[:, t * P:(t + 1) * P],
                        x16[t][:, hc * P:(hc + 1) * P],
                        ident[:],
                    )
                xt_t = xt_pool.tile([P, S], F16, tag="xT")
                nc.scalar.copy(xt_t[:], pt[:])
                xT.append(xt_t)

            # W^T slices (fp16): wT[(w, hc)] = [128, 384]
            wT = {}
            for wname in ("q", "k", "v"):
                w_ap = aps[f"w{wname}"]
                w16s = []
                for d in range(3):
                    wt_ = wload.tile([P, HIN], F32, tag="wload")
                    nc.sync.dma_start(wt_[:], w_ap[d * P:(d + 1) * P, :])
                    w16_t = w16_pool.tile(
                        [P, HIN], F16, tag="w16", name=f"w16{wname}_{d}"
                    )
                    nc.vector.tensor_copy(w16_t[:], wt_[:])
                    w16s.append(w16_t)
                for hc in range(6):
                    pw = psum1b.tile([P, 512], F16, tag="ps1b", name="pw")[:, :HOUT]
                    for d in range(3):
                        nc.tensor.transpose(
                            pw[:, d * P:(d + 1) * P],
                            w16s[d][:, hc * P:(hc + 1) * P],
                            ident[:],
                        )
                    wt_t = wt_pool.tile([P, HOUT], F16, tag="wT")
                    nc.scalar.copy(wt_t[:], pw[:])
                    wT[(wname, hc)] = wt_t

            # Q^T, K^T projections: fp16 matmuls, fp32 PSUM, fp32r output
            for wname, dest, scale in (("q", qT, 0.125), ("k", kT, 1.0)):
                for d in range(3):
                    for tch in range(2):
                        pp = psum1b.tile([P, 512], F32, tag="projp")
                        for hc in range(6):
                            nc.tensor.matmul(
                                pp[:],
                                wT[(wname, hc)][:, d * P:(d + 1) * P],
                                xT[hc][:, tch * 512:(tch + 1) * 512],
                                start=(hc == 0),
                                stop=(hc == 5),
                            )
                        nc.scalar.activation(
                            dest[d][:, tch * 512:(tch + 1) * 512],
                            pp[:],
                            AF.Identity,
                            bias=bias_sb[wname][:, d:d + 1],
                            scale=scale,
                        )

            # V projection: out [t-tile 128, 384] fp16 + ones column
            for t in range(8):
                pv = psum1b.tile([P, 512], F32, tag="projp", name="pv")[:, :HOUT]
                for hc in range(6):
                    nc.tensor.matmul(
                        pv[:],
                        xT[hc][:, t * P:(t + 1) * P],
                        wT[("v", hc)][:],
                        start=(hc == 0),
                        stop=(hc == 5),
                    )
                nc.vector.memset(v_tiles[t][:], 1.0)
                # copy + bias add (bv broadcast along partitions)
                nc.vector.tensor_add(
                    v_tiles[t][:, :, 0:HD],
                    pv[:].rearrange("p (h d) -> p h d", d=HD),
                    bv_bc[:],
                )

        # ---------------- phase 2: attention per head ----------------
        out_pool = ctx.enter_context(tc.tile_pool(name="outst", bufs=8))
        out_stage = [
            out_pool.tile([P, HOUT], F32, tag="outst", name=f"outst{i}")
            for i in range(8)
        ]
        pt_pool = ctx.enter_context(tc.tile_pool(name="pT", bufs=18))
        fin_pool = ctx.enter_context(tc.tile_pool(name="fin", bufs=4))
        spsum = ctx.enter_context(tc.tile_pool(name="spsum", bufs=4, space="PSUM"))
        vpsum = ctx.enter_context(tc.tile_pool(name="vpsum", bufs=4, space="PSUM"))
        ctt_pool = ctx.enter_context(tc.tile_pool(name="ctt", bufs=2))

        def emit_finalize(h, ctxT_ps):
            """Epilogue for head h: copy ctx^T out of PSUM, back-transpose to
            [q, 65], divide by the denominator. Deferred one head so the
            in-order PE stream never stalls waiting on the ACT copies."""
            ctxT_sb = [None, None]
            for qch in range(2):
                t_ = ctt_pool.tile([VW, 512], F32, tag="ctxT_sb",
                                   name=f"ctxTs{h}_{qch}")
                nc.scalar.copy(t_[:], ctxT_ps[qch][:])
                ctxT_sb[qch] = t_
            ctx_ps = [
                spsum.tile([P, 512], F32, tag="sT", name=f"ctx{h}_{i}")
                for i in range(2)
            ]
            # all PE transposes first, then all DVE reads: avoids the
            # per-slot PE-write/DVE-read same-bank ping-pong serialization
            for qt in range(QT):
                cp = ctx_ps[qt // 4]
                sl = (qt % 4) * VW
                nc.tensor.transpose(
                    cp[:, sl:sl + VW],
                    ctxT_sb[qt // 4][:, (qt % 4) * P:(qt % 4 + 1) * P],
                    ident32[:VW, :VW],
                )
            for qt in range(QT):
                cp = ctx_ps[qt // 4]
                sl = (qt % 4) * VW
                rc = fin_pool.tile([P, 1], F32, tag="recip")
                nc.vector.reciprocal(rc[:], cp[:, sl + HD:sl + HD + 1])
                nc.scalar.activation(
                    out_stage[qt][:, h * HD:(h + 1) * HD],
                    cp[:, sl:sl + HD],
                    AF.Identity,
                    scale=rc[:],
                )

        pending_fin = None
        for h in range(NH):
            # rel12 = rel1 + rel2 -> fp16, eight strip tiles [128, 1024]
            strips = []
            for qq in range(8):
                r1 = r1_pool.tile([P, S], F32, tag="r1")
                nc.sync.dma_start(
                    r1[:],
                    rel1_ap[h].rearrange("(qt p) k -> p qt k", p=P)[:, qq, :],
                )
                r2 = r2_pool.tile([P, S], F32, tag="r2")
                nc.sync.dma_start(
                    r2[:],
                    rel2_ap[h].rearrange("(qt p) k -> p qt k", p=P)[:, qq, :],
                )
                rb = rbf_pool.tile([P, S], F16, tag="rbf", name=f"rbf{h}_{qq}")
                nc.vector.tensor_add(rb[:], r1[:], r2[:])
                strips.append(rb)

            dt, rem = divmod(h, 2)
            d0 = rem * HD
            qTh = qT[dt][d0:d0 + HD, :]
            kTh = kT[dt][d0:d0 + HD, :]

            pT_strips = []
            for kt in range(KT):
                pT_strip = pt_pool.tile([P, S], F16, tag="pT", name=f"pT{h}_{kt}")
                pT_strips.append(pT_strip)
                for qch in range(2):
                    ps = spsum.tile([P, 512], F32, tag="sT")
                    # qk^T (fp32r: full-rate single-pass matmul)
                    nc.tensor.matmul(
                        ps[:],
                        kTh[:, kt * P:(kt + 1) * P],
                        qTh[:, qch * 512:(qch + 1) * 512],
                        start=True,
                        stop=False,
                    )
                    # += rel12^T (transposing adds via fp16 identity rhs)
                    for j in range(4):
                        qt = qch * 4 + j
                        nc.tensor.matmul(
                            ps[:, j * P:(j + 1) * P],
                            strips[qt][:, kt * P:(kt + 1) * P],
                            ident[:],
                            start=False,
                            stop=(j == 3),
                        )
                    # exp(scores + mask bias) -> fp16 probs
                    nc.scalar.activation(
                        pT_strip[:, qch * 512:(qch + 1) * 512],
                        ps[:],
                        AF.Exp,
                        bias=maskb[:, kt:kt + 1],
                        scale=1.0,
                    )
                if kt == 0 and pending_fin is not None:
                    emit_finalize(*pending_fin)
                    pending_fin = None

            # PV flipped: ctx^T[d|1, q] = V_aug.T @ P^T, accumulated over kt.
            # lhsT = V_aug tile (65 cols), rhs = pT strip (N=512 fp16) --
            # 16 big matmuls per head instead of 64 small ones. Row 64 of
            # ctx^T is the softmax denominator (ones column of V_aug).
            ctxT_ps = [
                vpsum.tile([VW, 512], F32, tag="ctxT", name=f"ctxT{h}_{i}")
                for i in range(2)
            ]
            for qch in range(2):
                for kt in range(KT):
                    nc.tensor.matmul(
                        ctxT_ps[qch][:],
                        v_tiles[kt][:, h, :],
                        pT_strips[kt][:, qch * 512:(qch + 1) * 512],
                        start=(kt == 0),
                        stop=(kt == KT - 1),
                    )
            pending_fin = (h, ctxT_ps)

        emit_finalize(*pending_fin)

        for qt in range(QT):
            nc.sync.dma_start(out_ap[qt * P:(qt + 1) * P, :], out_stage[qt][:])



def build_program():
    """Build and compile the per-core Bass program. Returns nc."""
    nc = bacc.Bacc(
        "TRN2",
        target_bir_lowering=False,
        debug=False,
        num_devices=8,
    )
    aps = {
        "x": nc.dram_tensor("x", [S, HIN], F32, kind="ExternalInput").ap(),
        "mask": nc.dram_tensor("mask", [S], I32, kind="ExternalInput").ap(),
        "rel1": nc.dram_tensor("rel1", [NH, S, S], F32, kind="ExternalInput").ap(),
        "rel2": nc.dram_tensor("rel2", [NH, S, S], F32, kind="ExternalInput").ap(),
        "wq": nc.dram_tensor("wq", [HOUT, HIN], F32, kind="ExternalInput").ap(),
        "wk": nc.dram_tensor("wk", [HOUT, HIN], F32, kind="ExternalInput").ap(),
        "wv": nc.dram_tensor("wv", [HOUT, HIN], F32, kind="ExternalInput").ap(),
        "bq": nc.dram_tensor("bq", [HOUT], F32, kind="ExternalInput").ap(),
        "bk": nc.dram_tensor("bk", [HOUT], F32, kind="ExternalInput").ap(),
        "bv": nc.dram_tensor("bv", [HOUT], F32, kind="ExternalInput").ap(),
        "out": nc.dram_tensor("out", [S, HOUT], F32, kind="ExternalOutput").ap(),
    }
    with tile.TileContext(nc) as tc:
        _build_kernel_body(tc, aps)
    nc.compile()
    return nc


def make_in_maps(inputs):
    """Slice full inputs into the 8 per-core input maps."""
    hs = np.ascontiguousarray(np.asarray(inputs["hidden_states"], np.float32))
    am = np.asarray(inputs["attention_mask"]).astype(np.int32)
    rel1 = np.asarray(inputs["rel_pos"], np.float32)
    rel2 = np.asarray(inputs["rel_2d_pos"], np.float32)
    ws = {k: np.asarray(inputs["W" + k[-1]], np.float32) for k in ("wq", "wk", "wv")}
    bs = {k: np.asarray(inputs["b" + k[-1]], np.float32) for k in ("bq", "bk", "bv")}

    in_maps = []
    for c in range(8):
        b, hh = divmod(c, 2)
        hsl = slice(hh * NH, (hh + 1) * NH)
        csl = slice(hh * HOUT, (hh + 1) * HOUT)
        m = {
            "x": np.ascontiguousarray(hs[b]),
            "mask": np.ascontiguousarray(am[b, 0, 0]),
            "rel1": np.ascontiguousarray(rel1[b, hsl]),
            "rel2": np.ascontiguousarray(rel2[b, hsl]),
        }
        for k in ("wq", "wk", "wv"):
            m[k] = np.ascontiguousarray(ws[k][csl])
        for k in ("bq", "bk", "bv"):
            m[k] = np.ascontiguousarray(bs[k][csl])
        in_maps.append(m)
    return in_maps


def gather_output(results):
    out = np.empty((4, S, HIN), np.float32)
    for c in range(8):
        b, hh = divmod(c, 2)
        out[b, :, hh * HOUT:(hh + 1) * HOUT] = results[c]["out"]
    return out


_NC_CACHE = []


def kernel(**inputs):
    if not _NC_CACHE:
        _NC_CACHE.append(build_program())
    nc = _NC_CACHE[0]
    in_maps = make_in_maps(inputs)
    res = run_bass_kernel_spmd(nc, in_maps, list(range(8)))
    return gather_output(res.results)
